# revision 1
# baseline (speedup 1.0000x reference)
"""CARAFE content-aware upsampling (S=2, K=5) as a Trainium2 Bass/Tile kernel.

v3: bf16 end-to-end, 4-chunk conv/softmax pipeline, pq-replicated tile-major
probability layout built with DVE 4x copies, PE warmup during DMA-in, split
input DMAs so the encoder starts early, balanced ACT/DVE copy assignment.

Sharding: 8 cores = 2 batches x 4 row-quarters (16 low-res rows each).
Per-core pipeline:
  1. content encoder 1x1 conv (PE, bf16): xc[64, 20*68] (3 col slices)
  2. per 4-row chunk c (4 chunks):
     kernel predictor 3x3 conv (9 shifted matmuls, N=256) -> kp_ps
     exp(+bias) ACT -> es[100, 256]; Z = selT.T @ es (PE); 1/Z (DVE)
     rep = selTT.T @ rz (PE); es *= rep (DVE in-place)
     4x DVE rep-copies -> p_rep[100, t*128 + pq*32 + pl] (pq-replicated)
  3. per tile-pair u (16 pairs):
     a. PE transpose x2 of p_rep 128-col slices -> PT[128=(pq,pl), 200]
     b. DVE copy -> SBUF; GPSIMD local_scatter -> mt[128, 240] (M^T)
     c. PE transpose x2 -> M[120, 256]; DVE copy -> SBUF
     d. PE matmul x2: o[128, 512] f32 = M.T @ patches
     e. ACT copy -> ost bf16; DMA out per 4 pairs.
Host prep (untimed): pad/slice x, bf16 convert, patch tiles, static tables.
"""

import os

os.environ.setdefault("MYCRO_LOCAL_CACHE", "1")

import numpy as np

import ml_dtypes
import concourse.bacc as bacc
import concourse.mybir as mybir
import concourse.tile as tile
from concourse.bass_utils import run_bass_kernel_spmd

F32 = mybir.dt.float32
BF16 = mybir.dt.bfloat16
I16 = mybir.dt.int16
U8 = mybir.dt.uint8
AF = mybir.ActivationFunctionType

B, C, H, W = 2, 256, 64, 64
S, K, COMP = 2, 5, 64
KP = 100          # S*S*K*K
K2 = 25
NCORES = 8
ROWS = 16         # low-res rows per core
RP, WPAD = 20, 68  # padded slice rows/cols
NTH, NTW = 8, 4   # tile grid: 8 x 4 tiles of 2x16 positions
NT = NTH * NTW    # 32 tiles
TPH, TPW = 2, 16  # tile position grid
POS = TPH * TPW   # 32
PATCH = 120       # (TPW+4) * (TPH+4) = 20*6, index = ww*6 + hh
GRID = RP * WPAD  # 1360
NCH = 4           # conv chunks (4 conv rows each)
CW = 256          # conv cols per chunk
RW = 4 * CW       # p_rep cols per chunk (pq-replicated)
MTW = 2 * PATCH   # 240
NPAIR = NT // 2   # 16
LAG = int(os.environ.get("K_LAG", "3"))
BUFS_C = int(os.environ.get("K_BUFS_C", "3"))
BUFS_PTM = int(os.environ.get("K_BUFS_PTM", "3"))
BUFS_O = int(os.environ.get("K_BUFS_O", "2"))
DMA_PER = int(os.environ.get("K_DMA_PER", "2"))   # pairs per out DMA (2 or 4)
OST_DVE_FROM = int(os.environ.get("K_OST_DVE_FROM", "10"))
NWU = int(os.environ.get("K_NWU", "5"))

# x DMA column splits of GRID (cover conv chunk needs progressively)
XSPLITS = [(0, 512), (512, 1024), (1024, GRID)]

# blob layout (bytes per partition)
OB_IDENT = 0                      # [128,128] bf16  -> 256B
OB_IDX = OB_IDENT + 256           # [128,200] i16   -> 400B
OB_WENC = OB_IDX + 400            # [128,2,64] bf16 -> 256B
OB_SELT = OB_WENC + 256           # [100,4] bf16    -> 8B
OB_SELTT = OB_SELT + 8            # [4,100] bf16    -> 200B
OB_BENC = OB_SELTT + 200          # [64,1] f32      -> 4B
OB_BKP = OB_BENC + 4              # [100,1] f32     -> 4B
NB = OB_BKP + 4


def _static_tables():
    # scatter: partition m = pq*32 + pl; src col j*100 + 4*k2 + pq
    # -> dst j*120 + (wp+b)*6 + (hp+a)
    idx = np.full((128, 2 * KP), -1, dtype=np.int16)
    for m in range(128):
        pq, pl = m // POS, m % POS
        hp, wp = pl // TPW, pl % TPW
        for j in range(2):
            for k2 in range(K2):
                a, b = k2 // K, k2 % K
                p = (wp + b) * 6 + (hp + a)
                idx[m, j * KP + 4 * k2 + pq] = j * PATCH + p
    selT = np.zeros((KP, 4), dtype=ml_dtypes.bfloat16)
    for ch in range(KP):
        selT[ch, ch % 4] = 1.0
    selTT = np.ascontiguousarray(selT.T)
    ident = np.eye(128, dtype=np.float32).astype(ml_dtypes.bfloat16)
    return idx, selT, selTT, ident


def build_kernel():
    nc = bacc.Bacc("TRN2", target_bir_lowering=False, debug=False)

    xs_c = nc.dram_tensor("xs_c", [C, GRID], BF16, kind="ExternalInput").ap()
    patches_d = nc.dram_tensor(
        "patches", [PATCH, NT * C], BF16, kind="ExternalInput"
    ).ap()
    w_kp9 = nc.dram_tensor("w_kp9", [COMP, 9 * KP], BF16, kind="ExternalInput").ap()
    blob_d = nc.dram_tensor("blob", [128, NB], U8, kind="ExternalInput").ap()
    out_d = nc.dram_tensor("out", [128, NT * C], BF16, kind="ExternalOutput").ap()

    with tile.TileContext(nc) as tc:
        _build(tc, nc, xs_c, patches_d, w_kp9, blob_d, out_d)
    nc.compile()
    return nc


def _build(tc, nc, xs_c, patches_d, w_kp9, blob_d, out_d):
    with (
        tc.tile_pool(name="const", bufs=1) as cpool,
        tc.tile_pool(name="work", bufs=1) as wpool,
        tc.tile_pool(name="cp", bufs=3) as cp,
        tc.tile_pool(name="pp", bufs=4) as pp,
        tc.tile_pool(name="opool", bufs=2) as opool,
        tc.tile_pool(name="ps_c", bufs=BUFS_C, space="PSUM") as ps_c,
        tc.tile_pool(name="ps_ptm", bufs=BUFS_PTM, space="PSUM") as ps_ptm,
        tc.tile_pool(name="ps_o", bufs=BUFS_O, space="PSUM") as ps_o,
    ):
        # ---- warmup source (no DMA dependency)
        wu_sb = cpool.tile([128, 512], BF16, tag="wu")
        nc.gpsimd.memset(wu_sb[:], 0.0)

        # ---- DMAs: blob+wkp on ACT queue; x slices + patches on SP queue
        blob_sb = cpool.tile([128, NB], U8, tag="blob")
        nc.scalar.dma_start(blob_sb[:], blob_d)
        wkp_sb = cpool.tile([COMP, 9 * KP], BF16, tag="wkp")
        nc.scalar.dma_start(wkp_sb[:], w_kp9)

        x_sb = cpool.tile([128, 2, GRID], BF16, tag="x")
        xg = xs_c.rearrange("(blk p) f -> p blk f", p=128)
        for lo, hi in XSPLITS:
            nc.sync.dma_start(x_sb[:, :, lo:hi], xg[:, :, lo:hi])
        pat_sb = cpool.tile([PATCH, NT * C], BF16, tag="pat")
        for lo, hi in ((0, NT * C // 2), (NT * C // 2, NT * C)):
            nc.sync.dma_start(pat_sb[:, lo:hi], patches_d[:, lo:hi])

        ident_sb = blob_sb[:, OB_IDENT:OB_IDENT + 256].bitcast(BF16)
        idx_sb = blob_sb[:, OB_IDX:OB_IDX + 400].bitcast(I16)
        wenc_sb = blob_sb[:, OB_WENC:OB_WENC + 256].bitcast(BF16).rearrange(
            "p (blk m) -> p blk m", blk=2)
        selT_sb = blob_sb[0:KP, OB_SELT:OB_SELT + 8].bitcast(BF16)
        selTT_sb = blob_sb[0:4, OB_SELTT:OB_SELTT + 200].bitcast(BF16)
        benc_sb = blob_sb[0:COMP, OB_BENC:OB_BENC + 4].bitcast(F32)
        bkp_sb = blob_sb[0:KP, OB_BKP:OB_BKP + 4].bitcast(F32)

        # ---- PE warmup: accumulating junk matmuls until real work arrives
        wu_ps = ps_c.tile([128, 512], F32, tag="c", name="wu_ps")
        for i in range(NWU):
            nc.tensor.matmul(wu_ps[:], wu_sb[:, 0:128], wu_sb[:],
                             start=(i == 0), stop=(i == NWU - 1))

        # ---- phase 1: encoder 1x1 conv -> xc [64, GRID] bf16
        xc_sb = wpool.tile([COMP, GRID], BF16, tag="xc")
        for si, (lo, hi) in enumerate(XSPLITS):
            enc_ps = ps_c.tile([COMP, hi - lo], F32, tag="c", name="enc_ps")
            for blk in range(2):
                nc.tensor.matmul(
                    enc_ps[:],
                    wenc_sb[:, blk, :],
                    x_sb[:, blk, lo:hi],
                    start=(blk == 0), stop=(blk == 1),
                )
            nc.scalar.activation(
                xc_sb[:, lo:hi], enc_ps[:], AF.Identity, bias=benc_sb)

        # ---- phases 2+3 per chunk -> p_rep (pq-replicated, tile-major)
        xc_g = xc_sb[:].rearrange("p (r w) -> p r w", r=RP)
        p_rep = wpool.tile([KP, NT * 128], BF16, tag="P")

        es_t = [None] * NCH
        rz_t = [None] * NCH
        rep_t = [None] * NCH

        def conv_front(c):
            kp_ps = ps_c.tile([KP, CW], F32, tag="c", name="kp_ps")
            r0 = 1 + 4 * c
            for tap in range(9):
                ti, tj = tap // 3, tap % 3
                rhs = xc_g[:, r0 + ti: r0 + ti + 4, 1 + tj: 65 + tj]
                nc.tensor.matmul(
                    kp_ps[:],
                    wkp_sb[:, tap * KP:(tap + 1) * KP],
                    rhs,
                    start=(tap == 0), stop=(tap == 8),
                )
            es_sb = cp.tile([KP, CW], BF16, tag="es")
            nc.scalar.activation(es_sb[:], kp_ps[:], AF.Exp, bias=bkp_sb)
            es_t[c] = es_sb

        def conv_zr(c):
            z_ps = ps_c.tile([4, CW], F32, tag="c", name="z_ps")
            nc.tensor.matmul(z_ps[:], selT_sb, es_t[c][:],
                             start=True, stop=True)
            rz_sb = cp.tile([4, CW], BF16, tag="rz")
            with nc.allow_low_precision(reason="recip feeds bf16 mults"):
                nc.vector.reciprocal(rz_sb[:], z_ps[:])
            rz_t[c] = rz_sb

        def conv_rmc(c):
            es_sb = es_t[c]
            rep_ps = ps_c.tile([KP, CW], F32, tag="c", name="rep_ps")
            nc.tensor.matmul(rep_ps[:], selTT_sb, rz_t[c][:],
                             start=True, stop=True)
            with nc.allow_low_precision(reason="softmax probs in bf16"):
                nc.vector.tensor_mul(es_sb[:], es_sb[:], rep_ps[:])
            # pq-replicated tile-major copies: one per (thl, hp);
            # dst col = (8c + thl*4 + tw)*128 + pq*32 + hp*16 + wp
            for thl in range(2):
                for hp in range(2):
                    src = es_sb[:, (thl * 2 + hp) * 64:(thl * 2 + hp + 1) * 64]
                    src = src.rearrange("p (tw wp) -> p tw wp", tw=4)
                    src = src.unsqueeze(2).broadcast_to([KP, 4, 4, TPW])
                    base = (8 * c + thl * 4) * 128
                    dst = p_rep[:, base: base + 512].rearrange(
                        "p (tw pq h w) -> p tw pq h w", tw=4, pq=4, h=2)
                    nc.vector.tensor_copy(dst[:, :, :, hp, :], src)

        # ---- phase 4: reassembly pairs, software pipeline
        mts = [None] * NPAIR
        ost = [None]

        def stage_a(u):
            t0 = 2 * u
            pt_ps = ps_ptm.tile([128, 2 * KP], BF16, tag="ptm", name="pt_ps")
            for j in range(2):
                t = t0 + j
                nc.tensor.transpose(
                    pt_ps[:, KP * j: KP * (j + 1)],
                    p_rep[:, 128 * t: 128 * (t + 1)],
                    ident_sb[0:KP, 0:KP])
            ptr_sb = pp.tile([128, 2 * KP], BF16, tag="ptr")
            nc.vector.tensor_copy(ptr_sb[:], pt_ps[:])
            mt_sb = pp.tile([128, MTW], BF16, tag="mt")
            nc.gpsimd.local_scatter(
                mt_sb[:], ptr_sb[:], idx_sb[:, 0: 2 * KP],
                channels=128, num_elems=MTW, num_idxs=2 * KP,
            )
            mts[u] = mt_sb

        def stage_b(u):
            t0 = 2 * u
            mt_sb = mts[u]
            m_ps = ps_ptm.tile([PATCH, 256], BF16, tag="ptm", name="m_ps")
            for j in range(2):
                nc.tensor.transpose(
                    m_ps[:, 128 * j: 128 * (j + 1)],
                    mt_sb[:, PATCH * j: PATCH * (j + 1)], ident_sb)
            m_sb = pp.tile([PATCH, 256], BF16, tag="msb")
            nc.vector.tensor_copy(m_sb[:], m_ps[:])
            if u % DMA_PER == 0:
                ost[0] = opool.tile(
                    [128, 2 * DMA_PER * C], BF16, tag="ost", name="ost_t")
            o_ps = ps_o.tile([128, 2 * C], F32, tag="o")
            for j in range(2):
                nc.tensor.matmul(
                    o_ps[:, C * j: C * (j + 1)],
                    m_sb[:, 128 * j: 128 * (j + 1)],
                    pat_sb[:, (t0 + j) * C:(t0 + j + 1) * C],
                    start=True, stop=True)
            w = (u % DMA_PER) * 2 * C
            if u >= OST_DVE_FROM and u % 2 == 1:
                nc.vector.tensor_copy(ost[0][:, w: w + 2 * C], o_ps[:])
            else:
                nc.scalar.copy(ost[0][:, w: w + 2 * C], o_ps[:])
            if u % DMA_PER == DMA_PER - 1:
                g = (u - (DMA_PER - 1)) * 2 * C
                nc.sync.dma_start(
                    out_d[:, g: g + 2 * DMA_PER * C], ost[0][:])

        # software-pipelined emission: keep the PE/DVE streams unblocked by
        # emitting each chunk's ladder ops interleaved with taps and pairs
        conv_front(0)
        conv_front(1)
        conv_zr(0)
        conv_front(2)
        conv_rmc(0)
        conv_zr(1)
        conv_front(3)
        conv_rmc(1)
        conv_zr(2)
        for step in range(NPAIR + LAG):
            if step == 2:
                conv_rmc(2)
                conv_zr(3)
            if step == 4:
                conv_rmc(3)
            if step < NPAIR:
                stage_a(step)
            if step >= LAG:
                stage_b(step - LAG)


def host_prep(x, w_enc, b_enc, w_kp, b_kp):
    """Build per-core input maps (pure relayout, untimed)."""
    idx, selT, selTT, ident = _static_tables()
    xpad = np.pad(x, ((0, 0), (0, 0), (2, 2), (2, 2)))  # [B, C, 68, 68]
    w_encT = np.ascontiguousarray(w_enc.T)              # [256, 64]
    w_kp9 = np.ascontiguousarray(
        np.transpose(w_kp, (1, 2, 3, 0)).reshape(COMP, 9 * KP)
    ).astype(ml_dtypes.bfloat16)

    blob = np.zeros((128, NB), np.uint8)
    blob[:, OB_IDENT:OB_IDENT + 256] = ident.view(np.uint8).reshape(128, 256)
    blob[:, OB_IDX:OB_IDX + 400] = idx.view(np.uint8).reshape(128, 400)
    wenc_b = w_encT.astype(ml_dtypes.bfloat16).reshape(2, 128, 64)
    wenc_b = np.ascontiguousarray(wenc_b.transpose(1, 0, 2)).reshape(128, 128)
    blob[:, OB_WENC:OB_WENC + 256] = wenc_b.view(np.uint8)
    blob[0:KP, OB_SELT:OB_SELT + 8] = np.ascontiguousarray(selT).view(
        np.uint8).reshape(KP, 8)
    blob[0:4, OB_SELTT:OB_SELTT + 200] = np.ascontiguousarray(selTT).view(
        np.uint8).reshape(4, 200)
    blob[0:COMP, OB_BENC:OB_BENC + 4] = np.ascontiguousarray(
        np.asarray(b_enc, np.float32).reshape(COMP, 1)).view(np.uint8)
    blob[0:KP, OB_BKP:OB_BKP + 4] = np.ascontiguousarray(
        np.asarray(b_kp, np.float32).reshape(KP, 1)).view(np.uint8)

    in_maps = []
    for core in range(NCORES):
        b, q = core // 4, core % 4
        sl = xpad[b, :, 16 * q: 16 * q + RP, :]          # [C, 20, 68]
        xs_c = np.ascontiguousarray(sl.reshape(C, GRID)).astype(
            ml_dtypes.bfloat16)
        # patch tiles: [PATCH, NT, C], p = ww*6 + hh
        pat = np.empty((NT, PATCH, C), dtype=ml_dtypes.bfloat16)
        for t in range(NT):
            th, tw = t // NTW, t % NTW
            blk = sl[:, 2 * th: 2 * th + 6, TPW * tw: TPW * tw + 20]  # [C,6,20]
            pat[t] = np.transpose(blk, (2, 1, 0)).reshape(PATCH, C)
        pat = np.ascontiguousarray(np.transpose(pat, (1, 0, 2)))
        in_maps.append({
            "xs_c": xs_c,
            "patches": pat.reshape(PATCH, NT * C),
            "w_kp9": w_kp9,
            "blob": blob,
        })
    return in_maps


def host_assemble(results):
    """results: list of 8 dicts with 'out' [128, NT*C] -> full [B, C, 128, 128]."""
    out = np.empty((B, C, H * S, W * S), dtype=np.float32)
    for core in range(NCORES):
        b, q = core // 4, core % 4
        # out rows m = pq*32 + hp*16 + wp, cols (t, c)
        a = results[core]["out"].astype(np.float32).reshape(
            2, 2, TPH, TPW, NTH, NTW, C)
        # dims: p, q2, hp, wp, th, tw, c -> [c, th, hp, p, tw, wp, q2]
        o = np.transpose(a, (6, 4, 2, 0, 5, 3, 1)).reshape(C, 32, 128)
        out[b, :, 32 * q: 32 * (q + 1), :] = o
    return out


_NC_CACHE = None


def kernel(x, w_enc, b_enc, w_kp, b_kp):
    global _NC_CACHE
    x = np.asarray(x)
    w_enc = np.asarray(w_enc)
    b_enc = np.asarray(b_enc)
    w_kp = np.asarray(w_kp)
    b_kp = np.asarray(b_kp)
    if _NC_CACHE is None:
        _NC_CACHE = build_kernel()
    nc = _NC_CACHE
    in_maps = host_prep(x, w_enc, b_enc, w_kp, b_kp)
    trace = os.environ.get("CARAFE_TRACE", "0") == "1"
    res = run_bass_kernel_spmd(nc, in_maps, list(range(NCORES)), trace=trace)
    out = host_assemble(res.results)
    if trace:
        kernel.last_exec_time_ns = res.exec_time_ns
        kernel.last_results = res
    return out



# revision 53
# speedup vs baseline: 1.3313x; 1.3313x over previous
"""CARAFE content-aware upsampling (S=2, K=5) as a Trainium2 Bass/Tile kernel.

v4: restructured reassembly — per (chunk, half) the exp/softmax tile is
PE-transposed once into position-major layout [128=(hp,tw,wp), 104], the
softmax normalization happens post-transpose as a tiny broadcast multiply,
and one GPSIMD local_scatter per half builds the 4-tile M^T [128, 4pq*120]
directly (no pq-replication copies, half the scatter traffic of v3).
Conv taps are packed in pairs via a row-shifted duplicate of xc living in
partitions 64-127 (6 matmuls per chunk instead of 9). Output matmuls write
bf16 PSUM and stage through one [128, 1024] copy per half before DMA.

Sharding: 8 cores = 2 batches x 4 row-quarters (16 low-res rows each).
Per-core pipeline:
  1. content encoder 1x1 conv (PE, bf16): xc -> dupA[0:64]; DVE copies
     build dupA[64:128] = xc shifted one grid row (+68 cols).
  2. per 4-row chunk c (4 chunks): kernel predictor 3x3 conv as 3 paired
     matmuls (contraction 128: taps (0,j)+(1,j)) + 3 single matmuls
     (taps (2,j)); exp(+bias) ACT -> es[0:100]; Z = selT.T @ es (PE);
     reciprocal -> es[100:104] (DVE).
  3. per half u=(c,thl) (8 units):
     a. PE transpose es[:, 128*thl:...] -> esT [128=(hp,tw,wp), 104]
     b. DVE copy -> SBUF; DVE broadcast mul normalizes cols 0:100
     c. GPSIMD local_scatter -> mt4[128, 480=(pq,p)]
     d. 4x PE transpose (per pq) -> m_ps [120, 512]; DVE reorder copy
        -> M_sb [120, (tw,pq,hp,wp)]
     e. 4x PE matmul (per tile): o_ps[128, 1024] bf16 = M.T @ patches
     f. ACT/DVE copy -> ost bf16; DMA out per half.
Host prep (untimed): pad/slice x, bf16 convert, patch tiles, static tables.
"""

import os

os.environ.setdefault("MYCRO_LOCAL_CACHE", "1")

import numpy as np

import ml_dtypes
import concourse.bacc as bacc
import concourse.mybir as mybir
import concourse.tile as tile
from concourse.bass_utils import run_bass_kernel_spmd

F32 = mybir.dt.float32
BF16 = mybir.dt.bfloat16
I16 = mybir.dt.int16
U8 = mybir.dt.uint8
AF = mybir.ActivationFunctionType

B, C, H, W = 2, 256, 64, 64
S, K, COMP = 2, 5, 64
KP = 100          # S*S*K*K
K2 = 25
NCORES = 8
ROWS = 16         # low-res rows per core
RP, WPAD = 20, 68  # padded slice rows/cols
NTH, NTW = 8, 4   # tile grid: 8 x 4 tiles of 2x16 positions
NT = NTH * NTW    # 32 tiles
TPH, TPW = 2, 16  # tile position grid
POS = TPH * TPW   # 32
PATCH = 120       # (TPW+4) * (TPH+4) = 20*6, index = ww*6 + hh
GRID = RP * WPAD  # 1360
NCH = 4           # conv chunks (4 conv rows each)
CW = 256          # conv cols per chunk
NU = 2 * NCH      # units: (chunk, thl)
MT4W = 4 * PATCH  # 480
NWU = int(os.environ.get("K_NWU", "4"))
B_C = int(os.environ.get("K_B_C", "2"))
B_A = int(os.environ.get("K_B_A", "1"))
B_M = int(os.environ.get("K_B_M", "2"))
B_O = int(os.environ.get("K_B_O", "3"))
B_MP = int(os.environ.get("K_B_MP", "3"))
ENC_POOL = int(os.environ.get("K_ENC_POOL", "4"))  # slices >= this on Pool
DIV_POOL = int(os.environ.get("K_DIV_POOL", "0"))  # softmax divide on Pool
E_TW = int(os.environ.get("K_E_TW", "1"))          # split E-copy by tile pair

# x DMA column splits of GRID / encoder compute slices
XSPLITS = [(0, 256), (256, 512), (512, 1024), (1024, GRID)]
ESLICES = XSPLITS

# blob0 layout (bytes per partition): needed at encoder start
OB_WENC = 0                       # [128,2,64] bf16 -> 256B
OB_BENC = OB_WENC + 256           # [64,1] f32      -> 4B
NB0 = OB_BENC + 4
# blob1 layout: needed from conv/reassembly
OB_IDENT = 0                      # [128,128] bf16  -> 256B
OB_IDX = OB_IDENT + 256           # [128,100] i16   -> 200B
OB_WPK = OB_IDX + 200             # [128,3,100] bf16 -> 600B (tap pairs)
OB_WS = OB_WPK + 600              # [64,3,100] bf16  -> 600B (taps (2,j))
OB_IHAT = OB_WS + 600             # [100,104] bf16  -> 208B (I | pq-sum)
OB_BKP = OB_IHAT + 208            # [100,1] f32     -> 4B
NB1 = OB_BKP + 4


def _static_tables():
    # scatter: partition m = hp*64 + tw*16 + wp; src col j = 4*k2 + pq
    # -> dst col pq*120 + (wp+b)*6 + (hp+a)
    idx = np.empty((128, KP), dtype=np.int16)
    for m in range(128):
        hp, wp = m // 64, m % 16
        for j in range(KP):
            k2, pq = j // 4, j % 4
            a, b = k2 // K, k2 % K
            idx[m, j] = pq * PATCH + (wp + b) * 6 + (hp + a)
    ident = np.eye(128, dtype=np.float32).astype(ml_dtypes.bfloat16)
    # A-matmul rhs: transpose identity plus per-pq channel-sum columns
    ihat = np.zeros((KP, 104), dtype=ml_dtypes.bfloat16)
    for ch in range(KP):
        ihat[ch, ch] = 1.0
        ihat[ch, KP + ch % 4] = 1.0
    return idx, ident, ihat


def build_kernel():
    nc = bacc.Bacc("TRN2", target_bir_lowering=False, debug=False)

    xs_c = nc.dram_tensor("xs_c", [C, GRID], BF16, kind="ExternalInput").ap()
    patches_d = nc.dram_tensor(
        "patches", [PATCH, NT * C], BF16, kind="ExternalInput"
    ).ap()
    blob0_d = nc.dram_tensor("blob0", [128, NB0], U8, kind="ExternalInput").ap()
    blob1_d = nc.dram_tensor("blob1", [128, NB1], U8, kind="ExternalInput").ap()
    out_d = nc.dram_tensor("out", [128, NT * C], BF16, kind="ExternalOutput").ap()

    with tile.TileContext(nc) as tc:
        _build(tc, nc, xs_c, patches_d, blob0_d, blob1_d, out_d)
    nc.compile()
    return nc


def _build(tc, nc, xs_c, patches_d, blob0_d, blob1_d, out_d):
    with (
        tc.tile_pool(name="const", bufs=1) as cpool,
        tc.tile_pool(name="work", bufs=1) as wpool,
        tc.tile_pool(name="cp", bufs=3) as cp,
        tc.tile_pool(name="ap", bufs=3) as ap,
        tc.tile_pool(name="mp", bufs=B_MP) as mp,
        tc.tile_pool(name="opool", bufs=3) as opool,
        tc.tile_pool(name="ps_c", bufs=B_C, space="PSUM") as ps_c,
        tc.tile_pool(name="ps_a", bufs=B_A, space="PSUM") as ps_a,
        tc.tile_pool(name="ps_m", bufs=B_M, space="PSUM") as ps_m,
        tc.tile_pool(name="ps_o", bufs=B_O, space="PSUM") as ps_o,
    ):
        # ---- warmup source (no DMA dependency)
        wu_sb = cpool.tile([128, 512], BF16, tag="wu")
        nc.gpsimd.memset(wu_sb[:], 0.0)

        # ---- DMAs: blob0 on ACT queue; blob1 + x slices + patches on SP
        blob0_sb = cpool.tile([128, NB0], U8, tag="blob0")
        nc.scalar.dma_start(blob0_sb[:], blob0_d)

        x_sb = cpool.tile([128, 2, GRID], BF16, tag="x")
        xg = xs_c.rearrange("(blk p) f -> p blk f", p=128)
        for lo, hi in XSPLITS[0:2]:
            nc.sync.dma_start(x_sb[:, :, lo:hi], xg[:, :, lo:hi])
        blob1_sb = cpool.tile([128, NB1], U8, tag="blob1")
        nc.sync.dma_start(blob1_sb[:], blob1_d)
        for lo, hi in XSPLITS[2:]:
            nc.sync.dma_start(x_sb[:, :, lo:hi], xg[:, :, lo:hi])
        pat_sb = cpool.tile([PATCH, NT * C], BF16, tag="pat")
        NQ = NT * C // 4
        for qi in range(4):
            nc.sync.dma_start(
                pat_sb[:, qi * NQ:(qi + 1) * NQ],
                patches_d[:, qi * NQ:(qi + 1) * NQ])

        wenc_sb = blob0_sb[:, OB_WENC:OB_WENC + 256].bitcast(BF16).rearrange(
            "p (blk m) -> p blk m", blk=2)
        benc_sb = blob0_sb[0:COMP, OB_BENC:OB_BENC + 4].bitcast(F32)
        ident_sb = blob1_sb[:, OB_IDENT:OB_IDENT + 256].bitcast(BF16)
        idx_sb = blob1_sb[:, OB_IDX:OB_IDX + 200].bitcast(I16)
        wpk_sb = blob1_sb[:, OB_WPK:OB_WPK + 600].bitcast(BF16).rearrange(
            "p (j m) -> p j m", j=3)
        ws_sb = blob1_sb[0:COMP, OB_WS:OB_WS + 600].bitcast(BF16).rearrange(
            "p (j m) -> p j m", j=3)
        ihat_sb = blob1_sb[0:KP, OB_IHAT:OB_IHAT + 208].bitcast(BF16)
        bkp_sb = blob1_sb[0:KP, OB_BKP:OB_BKP + 4].bitcast(F32)

        # ---- PE warmup: accumulating junk matmuls keep the p-state ramp
        # alive while DMAs land (emitted in spurts between real stages)
        def warm(n):
            wu_ps = ps_a.tile([128, KP], F32, tag="a", name="wu_ps")
            for i in range(n):
                nc.tensor.matmul(wu_ps[:], wu_sb[:, 0:128], wu_sb[:, 0:KP],
                                 start=(i == 0), stop=(i == n - 1))

        # ---- phase 1: encoder 1x1 conv -> dupA[0:64] = xc, then
        #      dupA[64:128, g] = xc[:, g+68] (one grid row down)
        dupA = wpool.tile([128, GRID], BF16, tag="xc")

        def enc_slice(si):
            lo, hi = ESLICES[si]
            enc_ps = ps_o.tile([COMP, 512], F32, tag="o", name="enc_ps")
            for blk in range(2):
                nc.tensor.matmul(
                    enc_ps[:, 0:hi - lo],
                    wenc_sb[:, blk, :],
                    x_sb[:, blk, lo:hi],
                    start=(blk == 0), stop=(blk == 1),
                )
            if si < ENC_POOL:
                nc.scalar.activation(
                    dupA[0:COMP, lo:hi], enc_ps[:, 0:hi - lo], AF.Identity,
                    bias=benc_sb)
            else:
                # keep the ACT queue clear for exp(c0): late slices' bias
                # adds run on the (idle-until-scatter) Pool engine
                with nc.allow_low_precision(reason="xc in bf16 as in v3"):
                    nc.gpsimd.tensor_tensor(
                        dupA[0:COMP, lo:hi], enc_ps[:, 0:hi - lo],
                        benc_sb.broadcast_to([COMP, hi - lo]),
                        op=mybir.AluOpType.add)
            dlo, dhi = max(lo - 68, 0), hi - 68
            nc.vector.tensor_copy(
                dupA[COMP:128, dlo:dhi], dupA[0:COMP, dlo + 68:hi])

        dupA_g = dupA[:].rearrange("p (r w) -> p r w", r=RP)

        # ---- per-chunk conv + softmax front, per-half reassembly
        es_t = [None] * NCH
        mt4_t = [None] * NU
        msb_t = [None] * NU
        ops_t = [None] * NU

        def conv(c):
            kp_ps = ps_c.tile([KP, CW], F32, tag="c", name="kp_ps")
            r0 = 1 + 4 * c
            for j in range(3):
                nc.tensor.matmul(
                    kp_ps[:],
                    wpk_sb[:, j, :],
                    dupA_g[:, r0:r0 + 4, 1 + j:65 + j],
                    start=(j == 0), stop=False,
                )
            for j in range(3):
                nc.tensor.matmul(
                    kp_ps[:],
                    ws_sb[:, j, :],
                    dupA_g[0:COMP, r0 + 2:r0 + 6, 1 + j:65 + j],
                    start=False, stop=(j == 2),
                )
            es_sb = cp.tile([KP, CW], BF16, tag="es")
            nc.scalar.activation(es_sb[:], kp_ps[:], AF.Exp, bias=bkp_sb)
            es_t[c] = es_sb

        def stage_a(u):
            # one matmul vs [I|pq-sum] -> pt_ps [128=(hp,tw,wp), 100 esT
            # cols + 4 Z cols]; fused divide+downconvert (DVE) -> prob_sb;
            # scatter into 4-tile M^T
            c, thl = u // 2, u % 2
            es_sb = es_t[c]
            pt_ps = ps_a.tile([128, 104], F32, tag="a", name="pt_ps")
            nc.tensor.matmul(
                pt_ps[:], es_sb[:, 128 * thl:128 * (thl + 1)], ihat_sb,
                start=True, stop=True)
            prob_sb = ap.tile([128, KP], BF16, tag="prob")
            prob = prob_sb[:].rearrange("p (k q) -> p k q", k=K2)
            src = pt_ps[:, 0:KP].rearrange("p (k q) -> p k q", k=K2)
            # DVE has no divide and may read only one non-scalar input from
            # PSUM: reciprocal the 4 Z columns into SBUF, then multiply
            rz_sb = ap.tile([128, 4], BF16, tag="zt")
            with nc.allow_low_precision(reason="recip feeds bf16 mults"):
                nc.vector.reciprocal(rz_sb[:], pt_ps[:, KP:104])
            zb = rz_sb[:].unsqueeze(1).broadcast_to([128, K2, 4])
            with nc.allow_low_precision(reason="softmax probs in bf16"):
                nc.vector.tensor_tensor(
                    prob, src, zb, op=mybir.AluOpType.mult)
            mt4 = mp.tile([128, MT4W], BF16, tag="mt4")
            nc.gpsimd.local_scatter(
                mt4[:], prob_sb[:], idx_sb[:, 0:KP],
                channels=128, num_elems=MT4W, num_idxs=KP,
            )
            mt4_t[u] = mt4

        def stage_b(u):
            # 4 per-pq transposes -> m_ps [120, (pq,hp,tw,wp)]; reorder copy
            # -> M_sb [120, (tw,pq,hp,wp)]
            mt4 = mt4_t[u]
            m_ps = ps_m.tile([PATCH, 512], BF16, tag="m", name="m_ps")
            for pq in range(4):
                nc.tensor.transpose(
                    m_ps[:, 128 * pq:128 * (pq + 1)],
                    mt4[:, PATCH * pq:PATCH * (pq + 1)], ident_sb)
            m_sb = mp.tile([PATCH, 512], BF16, tag="msb")
            src = m_ps[:].rearrange(
                "p (pq hp tw wp) -> p pq hp tw wp", pq=4, hp=2, tw=4)
            dst = m_sb[:].rearrange(
                "p (tw pq hp wp) -> p pq hp tw wp", tw=4, pq=4, hp=2)
            if E_TW:
                nc.vector.tensor_copy(dst[:, :, :, 0:2], src[:, :, :, 0:2])
                nc.scalar.copy(dst[:, :, :, 2:4], src[:, :, :, 2:4])
            else:
                nc.vector.tensor_copy(dst[:, 0:2], src[:, 0:2])
                nc.scalar.copy(dst[:, 2:4], src[:, 2:4])
            msb_t[u] = m_sb

        def stage_c(u):
            # 4 per-tile output matmuls -> 2x o_ps [128=(pq,hp,wp), 2*C] f32,
            # staged into ost bf16 (ACT first half, DVE second)
            m_sb = msb_t[u]
            t0 = 4 * u
            ost = opool.tile([128, 4 * C], BF16, tag="ost", name="ost_t")
            for half in range(2):
                o_ps = ps_o.tile([128, 2 * C], F32, tag="o", name="o_ps")
                for k in range(2):
                    tw = 2 * half + k
                    nc.tensor.matmul(
                        o_ps[:, C * k:C * (k + 1)],
                        m_sb[:, 128 * tw:128 * (tw + 1)],
                        pat_sb[:, (t0 + tw) * C:(t0 + tw + 1) * C],
                        start=True, stop=True)
                dst = ost[:, 2 * C * half:2 * C * (half + 1)]
                if half == 0:
                    nc.scalar.copy(dst, o_ps[:])
                else:
                    nc.vector.tensor_copy(dst, o_ps[:])
            ops_t[u] = ost

        def stage_d(u):
            ost = ops_t[u]
            g = 4 * u * C
            if u == NU - 1:
                # split the final DMA so the first half ships while the
                # second half's copy completes
                nc.sync.dma_start(out_d[:, g:g + 2 * C], ost[:, 0:2 * C])
                nc.sync.dma_start(
                    out_d[:, g + 2 * C:g + 4 * C], ost[:, 2 * C:4 * C])
            else:
                nc.sync.dma_start(out_d[:, g:g + 4 * C], ost[:])

        # software-pipelined emission: keep PE stream dense; stage_a(u)
        # needs conv(u//2); stage_c needs stage_b needs stage_a. Warmup
        # spurts bridge PE idle gaps during the DMA-bound preamble so the
        # p-state ramp survives into the real work.
        warm(4)
        enc_slice(0)
        enc_slice(1)
        warm(2)
        conv(0)
        enc_slice(2)
        stage_a(0)
        stage_a(1)
        enc_slice(3)
        conv(1)
        stage_b(0)
        conv(2)
        stage_a(2)
        stage_b(1)
        stage_c(0)
        stage_d(0)
        conv(3)
        stage_a(3)
        stage_b(2)
        stage_c(1)
        stage_d(1)
        stage_a(4)
        stage_b(3)
        stage_c(2)
        stage_d(2)
        stage_a(5)
        stage_b(4)
        stage_c(3)
        stage_d(3)
        stage_a(6)
        stage_b(5)
        stage_c(4)
        stage_d(4)
        stage_a(7)
        stage_b(6)
        stage_c(5)
        stage_d(5)
        stage_b(7)
        stage_c(6)
        stage_d(6)
        stage_c(7)
        stage_d(7)


def host_prep(x, w_enc, b_enc, w_kp, b_kp):
    """Build per-core input maps (pure relayout, untimed)."""
    idx, ident, ihat = _static_tables()
    xpad = np.pad(x, ((0, 0), (0, 0), (2, 2), (2, 2)))  # [B, C, 68, 68]
    w_encT = np.ascontiguousarray(w_enc.T)              # [256, 64]
    w_kp9 = np.ascontiguousarray(
        np.transpose(w_kp, (1, 2, 3, 0)).reshape(COMP, 9 * KP)
    ).astype(ml_dtypes.bfloat16)

    blob0 = np.zeros((128, NB0), np.uint8)
    wenc_b = w_encT.astype(ml_dtypes.bfloat16).reshape(2, 128, COMP)
    wenc_b = np.ascontiguousarray(wenc_b.transpose(1, 0, 2)).reshape(128, 128)
    blob0[:, OB_WENC:OB_WENC + 256] = wenc_b.view(np.uint8)
    blob0[0:COMP, OB_BENC:OB_BENC + 4] = np.ascontiguousarray(
        np.asarray(b_enc, np.float32).reshape(COMP, 1)).view(np.uint8)

    blob1 = np.zeros((128, NB1), np.uint8)
    blob1[:, OB_IDENT:OB_IDENT + 256] = ident.view(np.uint8).reshape(128, 256)
    blob1[:, OB_IDX:OB_IDX + 200] = idx.view(np.uint8).reshape(128, 200)
    # conv tap pairs: rows 0:64 tap (0,j), rows 64:128 tap (1,j)
    wpk = np.zeros((128, 3, KP), dtype=ml_dtypes.bfloat16)
    for j in range(3):
        wpk[0:COMP, j] = w_kp9[:, j * KP:(j + 1) * KP]
        wpk[COMP:128, j] = w_kp9[:, (3 + j) * KP:(4 + j) * KP]
    blob1[:, OB_WPK:OB_WPK + 600] = wpk.reshape(128, 300).view(np.uint8)
    wsing = np.zeros((COMP, 3, KP), dtype=ml_dtypes.bfloat16)
    for j in range(3):
        wsing[:, j] = w_kp9[:, (6 + j) * KP:(7 + j) * KP]
    blob1[0:COMP, OB_WS:OB_WS + 600] = wsing.reshape(COMP, 300).view(np.uint8)
    blob1[0:KP, OB_IHAT:OB_IHAT + 208] = np.ascontiguousarray(ihat).view(
        np.uint8).reshape(KP, 208)
    blob1[0:KP, OB_BKP:OB_BKP + 4] = np.ascontiguousarray(
        np.asarray(b_kp, np.float32).reshape(KP, 1)).view(np.uint8)

    in_maps = []
    for core in range(NCORES):
        b, q = core // 4, core % 4
        sl = xpad[b, :, 16 * q: 16 * q + RP, :]          # [C, 20, 68]
        xs_c = np.ascontiguousarray(sl.reshape(C, GRID)).astype(
            ml_dtypes.bfloat16)
        # patch tiles: [PATCH, NT, C], p = ww*6 + hh
        pat = np.empty((NT, PATCH, C), dtype=ml_dtypes.bfloat16)
        for t in range(NT):
            th, tw = t // NTW, t % NTW
            blk = sl[:, 2 * th: 2 * th + 6, TPW * tw: TPW * tw + 20]  # [C,6,20]
            pat[t] = np.transpose(blk, (2, 1, 0)).reshape(PATCH, C)
        pat = np.ascontiguousarray(np.transpose(pat, (1, 0, 2)))
        in_maps.append({
            "xs_c": xs_c,
            "patches": pat.reshape(PATCH, NT * C),
            "blob0": blob0,
            "blob1": blob1,
        })
    return in_maps


def host_assemble(results):
    """results: list of 8 dicts with 'out' [128, NT*C] -> full [B, C, 128, 128]."""
    out = np.empty((B, C, H * S, W * S), dtype=np.float32)
    for core in range(NCORES):
        b, q = core // 4, core % 4
        # out rows m = pq*32 + hp*16 + wp, cols (t, c)
        a = results[core]["out"].astype(np.float32).reshape(
            2, 2, TPH, TPW, NTH, NTW, C)
        # dims: p, q2, hp, wp, th, tw, c -> [c, th, hp, p, tw, wp, q2]
        o = np.transpose(a, (6, 4, 2, 0, 5, 3, 1)).reshape(C, 32, 128)
        out[b, :, 32 * q: 32 * (q + 1), :] = o
    return out


_NC_CACHE = None


def kernel(x, w_enc, b_enc, w_kp, b_kp):
    global _NC_CACHE
    x = np.asarray(x)
    w_enc = np.asarray(w_enc)
    b_enc = np.asarray(b_enc)
    w_kp = np.asarray(w_kp)
    b_kp = np.asarray(b_kp)
    if _NC_CACHE is None:
        _NC_CACHE = build_kernel()
    nc = _NC_CACHE
    in_maps = host_prep(x, w_enc, b_enc, w_kp, b_kp)
    trace = os.environ.get("CARAFE_TRACE", "0") == "1"
    res = run_bass_kernel_spmd(nc, in_maps, list(range(NCORES)), trace=trace)
    out = host_assemble(res.results)
    if trace:
        kernel.last_exec_time_ns = res.exec_time_ns
        kernel.last_results = res
    return out
